# revision 57
# baseline (speedup 1.0000x reference)
"""Causal multi-head attention block (QKV proj + causal softmax attention + out proj)
for Trainium2, sharded over 8 NeuronCores: data-parallel over batch (2), tensor-
parallel over heads (16 heads -> 4 per core).

Shapes (hardcoded): B=2, T=2048, C=1024, H=16, Dh=64.
Each core computes a partial output projection [T, C] for its 4 heads; the host
sums the 4 partials per batch and adds the fc bias.

Schedule: per t-chunk, score matmuls are emitted at the top of each pair slot
(so the scalar engine's Exp stream never starves), head pairs alternate row
groups for PE-array concurrency, and QKV-of-next-chunk / FC-of-previous-chunk
matmul groups are drip-fed as PE filler between pairs to keep the tensor
engine dense (HAM stays at 8/8). Softmax normalization broadcasts the
reciprocal denominator row through a K=1 ones-matmul into the upper half of
the same AV PSUM bank (no DRAM bounce).
"""

import os

import numpy as np

import concourse.bass as bass
import concourse.tile as tile
from concourse import bacc, mybir
from concourse.bass_utils import run_bass_kernel_spmd

F32 = mybir.dt.float32
BF16 = mybir.dt.bfloat16

B = 2
T = 2048
C = 1024
H_PER_CORE = 4  # local heads per core
DH = 64
O_CORE = H_PER_CORE * DH  # 256 output channels per core (per q/k/v)

TCH = 512  # t-chunk size (free dim of most matmuls)
N_CHUNKS = T // TCH  # 4
KT = T // 128  # 16 k-tiles of 128

_BUILD_CACHE = {}
LAST_RESULT = None


def build(t=T):
    n_chunks = t // TCH
    nc = bacc.Bacc("TRN2", target_bir_lowering=False)

    xT = nc.declare_dram_parameter("xT", [C, t], BF16, isOutput=False)
    # host-shuffled qkv weights: per output-block contiguous, [ci, blk(co, o)]
    # blocks: q0/q1/k0/k1 (128 cols each), v (256 cols)
    wqkvS = nc.declare_dram_parameter("wqkvS", [128, 8 * 3 * O_CORE], BF16, isOutput=False)
    bqk = nc.declare_dram_parameter("bqk", [128, 4], F32, isOutput=False)
    bv_rep = nc.declare_dram_parameter("bv_rep", [128, O_CORE], F32, isOutput=False)
    wfcT = nc.declare_dram_parameter("wfcT", [O_CORE, C], BF16, isOutput=False)
    mask = nc.declare_dram_parameter("mask", [128, 128], BF16, isOutput=False)
    y = nc.declare_dram_parameter("y", [t, C], BF16, isOutput=True)
    # ks1-partial of the last chunk's FC (host adds it into y's last rows);
    # splitting lets the ks0 half run as PE filler before the final epilogue
    y2 = nc.declare_dram_parameter("y2", [TCH, C], BF16, isOutput=True)

    with (
        tile.TileContext(nc) as tc,
        tc.tile_pool(name="singles", bufs=1) as singles,
        tc.tile_pool(name="xpool", bufs=3) as xpool,
        tc.tile_pool(name="wtpool", bufs=8) as wtpool,
        tc.tile_pool(name="attnpool", bufs=4) as attnpool,
        tc.tile_pool(name="opool", bufs=3) as opool,
        tc.tile_pool(name="rpool", bufs=4) as rpool,
        tc.tile_pool(name="dpool", bufs=4, space="DRAM") as dpool,
        tc.tile_pool(name="mmps", bufs=2, space="PSUM") as mmps,
        tc.tile_pool(name="sps", bufs=2, space="PSUM") as sps,
        tc.tile_pool(name="avps", bufs=2, space="PSUM") as avps,
    ):
        xT_r = xT.rearrange("(co ci) t -> ci co t", ci=128)

        # ---- startup loads, split across the two DMA-issuing queues in
        # critical-path order: x chunk0 + q0/k0 first, everything else after.
        # The qkv weights are host-shuffled so each output block is one
        # contiguous DMA into its own tile (exact per-block deps)
        offs = [0, 1024, 2048, 3072, 4096, 6144]
        wqB = [None] * 5

        def load_wq(bi, eng):
            w = (offs[bi + 1] - offs[bi]) // 8
            tile_b = singles.tile([128, 8, w], BF16, name=f"wqB{bi}")
            eng.dma_start(
                tile_b[:],
                wqkvS[:, offs[bi] : offs[bi + 1]].rearrange("p (co o) -> p co o", co=8),
            )
            wqB[bi] = tile_b

        xt0 = xpool.tile([128, 8, TCH], BF16, tag="xt", name="xt0")
        nc.sync.dma_start(xt0[:, 0:4, :], xT_r[:, 0:4, 0:TCH])
        nc.gpsimd.dma_start(xt0[:, 4:8, :], xT_r[:, 4:8, 0:TCH])
        load_wq(0, nc.sync)  # q0
        load_wq(2, nc.gpsimd)  # k0
        bqk_sb = singles.tile([128, 4], F32)
        nc.sync.dma_start(bqk_sb[:], bqk[:])
        load_wq(1, nc.gpsimd)  # q1
        load_wq(3, nc.sync)  # k1
        load_wq(4, nc.gpsimd)  # v
        mask_sb = singles.tile([128, 128], BF16)
        nc.sync.dma_start(mask_sb[:], mask[:])
        bv_sb = singles.tile([128, H_PER_CORE, DH], F32)
        nc.sync.dma_start(bv_sb[:], bv_rep.rearrange("p (h d) -> p h d", h=H_PER_CORE))
        wfc_sb = singles.tile([128, 2, C], BF16)  # [p, ks, n]
        nc.gpsimd.dma_start(wfc_sb[:], wfcT.rearrange("(ks p) n -> p ks n", p=128))

        # PE warmup: dummy matmuls fill the startup DMA wait so HAM reaches
        # 8/8 before the first real matmul; results go to psum tiles that are
        # never read
        scr = singles.tile([128, TCH], BF16)
        nc.vector.memset(scr[:], 0.0)
        for wi in range(16):
            wps = mmps.tile([128, TCH], F32, tag="mm", name="warmps")
            nc.tensor.matmul(wps[:], scr[:, 0:128], scr[:], start=True, stop=True)
        ones_sb = singles.tile([1, DH], BF16)
        nc.vector.memset(ones_sb[:], 1.0)

        qT_sb = singles.tile([128, 2, t], BF16)  # [dh + 64*(h%2), h//2, t]
        kT_sb = singles.tile([128, 2, t], BF16)
        v_sb = singles.tile([128, t // 128, H_PER_CORE, DH + 1], BF16)  # [k_in, kt, h, d|1]
        nc.vector.memset(v_sb[:, :, :, DH : DH + 1], 1.0)  # ones col -> softmax denom

        def load_xt(tcix):
            ts0 = tcix * TCH
            xt = xpool.tile([128, 8, TCH], BF16, tag="xt", name=f"xt{tcix}")
            nc.sync.dma_start(xt[:, 0:4, :], xT_r[:, 0:4, ts0 : ts0 + TCH])
            nc.gpsimd.dma_start(xt[:, 4:8, :], xT_r[:, 4:8, ts0 : ts0 + TCH])
            return xt

        def qkv_units(tcix, xt):
            ts0 = tcix * TCH

            def qk_group(i):
                ps = mmps.tile([128, TCH], F32, tag="mm", name="qkps")
                for co in range(8):
                    nc.tensor.matmul(
                        ps[:],
                        wqB[i][:, co, :],
                        xt[:, co, :],
                        start=(co == 0),
                        stop=(co == 7),
                    )
                dst = qT_sb if i < 2 else kT_sb
                nc.vector.tensor_scalar_add(
                    dst[:, i % 2, ts0 : ts0 + TCH], ps[:], bqk_sb[:, i : i + 1]
                )

            def v_group(j):
                ps = mmps.tile([128, TCH], F32, tag="mm", name="vps")
                for half in range(2):
                    tt = j * 2 + half
                    for co in range(8):
                        nc.tensor.matmul(
                            ps[:, half * 256 : (half + 1) * 256],
                            xt[:, co, tt * 128 : (tt + 1) * 128],
                            wqB[4][:, co, :],
                            start=(co == 0),
                            stop=(co == 7),
                        )
                kt0 = tcix * 4 + j * 2
                nc.vector.tensor_add(
                    v_sb[:, kt0 : kt0 + 2, :, 0:DH],
                    ps.rearrange("p (a h d) -> p a h d", a=2, h=H_PER_CORE),
                    bv_sb[:, None, :, :].to_broadcast((128, 2, H_PER_CORE, DH)),
                )

            # q0/k0 first so chunk-0 attention can start earliest
            return [
                lambda: qk_group(0),
                lambda: qk_group(2),
                lambda: qk_group(1),
                lambda: qk_group(3),
                lambda: v_group(0),
                lambda: v_group(1),
            ]

        def fc_units(ts0, attn_t):
            def unit(tt):
                ot = opool.tile([128, 2, TCH], BF16, tag="o")
                for nn in range(2):
                    ps = mmps.tile([128, TCH], F32, tag="mm", name="fcps")
                    for ks in range(2):
                        nc.tensor.matmul(
                            ps[:],
                            attn_t[:, ks, tt * 128 : (tt + 1) * 128],
                            wfc_sb[:, ks, nn * TCH : (nn + 1) * TCH],
                            start=(ks == 0),
                            stop=(ks == 1),
                        )
                    nc.vector.tensor_copy(ot[:, nn, :], ps[:])
                eng = nc.gpsimd if tt % 2 == 0 else nc.sync
                eng.dma_start(
                    y[ts0 + tt * 128 : ts0 + (tt + 1) * 128, :].rearrange(
                        "p (a q) -> p a q", a=2
                    ),
                    ot[:],
                )

            return [lambda tt=tt: unit(tt) for tt in range(4)]

        def fc_half_units(ts0, attn_t, ks, ydst):
            """Single-ks FC partials for the last chunk (no psum accumulation
            across ks, so the ks0 half can run before the hp1 epilogue)."""

            def unit(tt):
                ot = opool.tile([128, 2, TCH], BF16, tag="o")
                for nn in range(2):
                    ps = mmps.tile([128, TCH], F32, tag="mm", name="fchps")
                    nc.tensor.matmul(
                        ps[:],
                        attn_t[:, ks, tt * 128 : (tt + 1) * 128],
                        wfc_sb[:, ks, nn * TCH : (nn + 1) * TCH],
                        start=True,
                        stop=True,
                    )
                    nc.vector.tensor_copy(ot[:, nn, :], ps[:])
                eng = nc.gpsimd if tt % 2 == 0 else nc.sync
                eng.dma_start(
                    ydst[ts0 + tt * 128 : ts0 + (tt + 1) * 128, :].rearrange(
                        "p (a q) -> p a q", a=2
                    ),
                    ot[:],
                )

            return [lambda tt=tt: unit(tt) for tt in range(4)]

        qkvq = []  # QKV filler units; must drain before their chunk starts
        fcfill = []  # FC filler units; may roll forward
        fcq = []  # finished chunks awaiting FC
        tail_work = []  # deferred final-epilogue stages (PE bcast/copy/mul)

        def pop_filler(reserve=0):
            if qkvq:
                qkvq.pop(0)()
            elif len(fcfill) > reserve:
                fcfill.pop(0)()

        for tcix in range(n_chunks):
            ts0 = tcix * TCH
            if tcix + 1 < n_chunks:
                qkvq += qkv_units(tcix + 1, load_xt(tcix + 1))
            if tcix == 0:
                for u in qkv_units(0, xt0):
                    u()
            # FC filler plan (balances ACT-bound later chunks): chunk3 gets
            # FC(0)+FC(1)+FC(2); FC(3) runs at the tail.
            if tcix == 3:
                while fcq:
                    fcfill += fc_units(*fcq.pop(0))

            attn_t = attnpool.tile([128, 2, TCH], BF16, tag="attn")
            kimax = tcix * 4 + 3
            npairs = 2 * tcix + 2
            for hp in range(2):
                heads = (2 * hp, 2 * hp + 1)
                ps_av = {
                    h: avps.tile([128, TCH], F32, tag="av", name=f"av{h}")
                    for h in heads
                }
                wts = {}
                pend = []

                def do_av(item):
                    h, pj = item
                    wt = wts.pop((h, pj))
                    for u in range(2):
                        ki = 2 * pj + u
                        sx = max(0, (ki - 4 * tcix) * 128)
                        nc.tensor.matmul(
                            ps_av[h][0 : DH + 1, sx:TCH],
                            v_sb[:, ki, h, :],
                            wt[:, u, sx:TCH],
                            start=(ki == 0),
                            stop=(ki == kimax),
                            skip_group_check=True,
                        )

                for pj in range(npairs):
                    m0 = 2 * pj - 4 * tcix  # block offset of the pair's first ki
                    sts = {
                        h: sps.tile([128, 2, TCH], F32, tag="s", name=f"s{h}")
                        for h in heads
                    }
                    # scores: u-major, heads adjacent -> the two heads occupy
                    # disjoint PE row groups and stream concurrently
                    for u in range(2):
                        ki = 2 * pj + u
                        m = ki - 4 * tcix
                        # partial blocks: columns < sx are never consumed
                        # downstream (they feed an unread, bounded exp), so
                        # skip streaming the dead columns
                        sx = 128 * m if m >= 1 else 0
                        for h in heads:
                            pb = (h % 2) * 64
                            ho = h // 2
                            nc.tensor.matmul(
                                sts[h][:, u, sx:],
                                kT_sb[pb : pb + 64, ho, ki * 128 : (ki + 1) * 128],
                                qT_sb[pb : pb + 64, ho, ts0 + sx : ts0 + TCH],
                                start=True,
                                stop=True,
                                skip_group_check=True,
                            )
                    for h in heads:
                        wt = wtpool.tile([128, 2, TCH], BF16, tag="wt")
                        # mostly-masked final pair: exp only from the first live
                        # column (u=1's stale prefix feeds an unread, bounded exp)
                        sxm = 128 * m0 if m0 >= 2 else 0
                        nc.scalar.activation(
                            wt[:, :, sxm:TCH],
                            sts[h][:, :, sxm:TCH],
                            mybir.ActivationFunctionType.Exp,
                            scale=0.125,
                        )
                        for u in range(2):
                            ki = 2 * pj + u
                            m = ki - 4 * tcix
                            if m >= 0:
                                sx = m * 128
                                nc.vector.tensor_mul(
                                    wt[:, u, sx : sx + 128],
                                    wt[:, u, sx : sx + 128],
                                    mask_sb[:],
                                )
                        wts[(h, pj)] = wt
                        pend.append((h, pj))
                    # on the last chunk hold back 6 filler units: they keep the
                    # PE busy through the final epilogue's latency chain
                    pop_filler(reserve=6 if tcix == n_chunks - 1 else 0)
                    while len(pend) > 6:
                        do_av(pend.pop(0))
                if final_drain := (tcix == n_chunks - 1 and hp == 1):
                    # drain the first head's AVs before the second head's so
                    # its epilogue chain overlaps the remaining AV matmuls
                    pend.sort(key=lambda it: (it[0], it[1]))
                while pend:
                    do_av(pend.pop(0))

                # softmax normalize: fast-approx reciprocal of the denom row
                # (~18 bits, 5x faster than reciprocal), broadcast across 64
                # partitions, then one multiply; the two heads' chains overlap.
                # Mid-kernel the broadcast is a 2-hop DRAM bounce (latency is
                # hidden); the FINAL epilogue instead uses a K=1 ones-matmul
                # into the AV bank's upper half + scalar-engine copy-out, which
                # is much shorter on the exposed tail path.
                final_ep = tcix == n_chunks - 1 and hp == 1
                reps = {}
                for h in heads:
                    # stage the denom row to SBUF partition 0 first: the custom-
                    # DVE reciprocal needs a partition-0 source
                    drow = rpool.tile([1, TCH], F32, tag="drow", name=f"drow{h}")
                    nc.vector.tensor_copy(drow[:], ps_av[h][DH : DH + 1, :])
                    rcp = rpool.tile([1, TCH], F32, tag="rcp", name=f"rcp{h}")
                    nc.vector.reciprocal_approx_fast(rcp[:], drow[:])
                    rep = rpool.tile([64, TCH], F32, tag="rep", name=f"rep{h}")
                    if final_ep:
                        # PE-broadcast path, with the PE/ACT/DVE stages deferred
                        # to the tail so reserved filler can run ahead of them
                        rcpb = rpool.tile([1, TCH], BF16, tag="rcpb", name=f"rcpb{h}")
                        nc.scalar.copy(rcpb[:], rcp[:])

                        def fin(h=h, rcpb=rcpb, rep=rep, ps=ps_av[h], at=attn_t):
                            nc.tensor.matmul(
                                ps[64:128, :],
                                ones_sb[:],
                                rcpb[:],
                                start=True,
                                stop=True,
                                skip_group_check=True,
                            )
                            nc.scalar.copy(rep[:], ps[64:128, :])
                            pb = (h % 2) * 64
                            ho = h // 2
                            nc.vector.tensor_mul(
                                at[pb : pb + 64, ho, :], ps[0:DH, :], rep[:]
                            )

                        tail_work.append(fin)
                    else:
                        d1 = dpool.tile([1, TCH], F32)
                        nc.sync.dma_start(d1[:], rcp[:])
                        # same queue as the d1 write: in-order issue is what
                        # sequences the bounce (the raw-AP read is not tracked)
                        nc.sync.dma_start(
                            rep[:],
                            bass.AP(
                                tensor=d1.tensor,
                                offset=d1.offset,
                                ap=[[0, 64], [1, TCH]],
                            ),
                        )
                    reps[h] = rep
                if not final_ep:
                    for h in heads:
                        pb = (h % 2) * 64
                        ho = h // 2
                        nc.vector.tensor_mul(
                            attn_t[pb : pb + 64, ho, :],
                            ps_av[h][0:DH, :],
                            reps[h][:],
                        )
                if tcix == n_chunks - 1 and hp == 0:
                    # last chunk: the ks0 FC half only needs hp0's heads ->
                    # becomes PE filler during hp1's (ACT-bound) attention
                    fcfill += fc_half_units(ts0, attn_t, 0, y)
            fcq.append((ts0, attn_t))
            # QKV for chunk tcix+1 must be fully emitted before its attention
            while qkvq:
                qkvq.pop(0)()

        # tail: reserved filler covers the final-epilogue latency; the deferred
        # epilogue stages land between filler units, then the ks1 FC half
        # with stores merged into two double-width DMAs
        ts_last, attn_last = fcq.pop()
        for _ in range(3):
            if fcfill:
                fcfill.pop(0)()
        for w in tail_work:
            w()
        while fcfill:
            fcfill.pop(0)()
        for half in range(2):
            ot = opool.tile([128, 2, 2, TCH], BF16, tag="o2", name=f"otl{half}")
            for ti in range(2):
                tt = half * 2 + ti
                for nn in range(2):
                    ps = mmps.tile([128, TCH], F32, tag="mm", name="fclps")
                    nc.tensor.matmul(
                        ps[:],
                        attn_last[:, 1, tt * 128 : (tt + 1) * 128],
                        wfc_sb[:, 1, nn * TCH : (nn + 1) * TCH],
                        start=True,
                        stop=True,
                    )
                    nc.vector.tensor_copy(ot[:, ti, nn, :], ps[:])
            eng = nc.gpsimd if half == 0 else nc.sync
            eng.dma_start(
                y2[half * 256 : (half + 1) * 256, :].rearrange(
                    "(a p) (b q) -> p a b q", p=128, b=2
                ),
                ot[:],
            )

    nc.compile()
    return nc


def _prep_core_inputs(x, w_qkv, b_qkv, w_fc, b_fc, core):
    b, g = core // 4, core % 4
    rq = slice(256 * g, 256 * g + 256)
    rk = slice(1024 + 256 * g, 1024 + 256 * g + 256)
    rv = slice(2048 + 256 * g, 2048 + 256 * g + 256)
    wcat = np.concatenate([w_qkv[rq], w_qkv[rk], w_qkv[rv]], axis=0)  # [768, 1024]
    bq, bk, bv = b_qkv[rq], b_qkv[rk], b_qkv[rv]
    import ml_dtypes

    bf16 = ml_dtypes.bfloat16
    # per-block contiguous shuffle: [ci, blk(co, o)] with blocks
    # q0/q1/k0/k1 (128 wide) then v (256 wide)
    pieces = []
    for s0, s1 in [(0, 128), (128, 256), (256, 384), (384, 512), (512, 768)]:
        p = wcat[s0:s1].T  # [1024 (co ci), w]
        w = s1 - s0
        p = p.reshape(8, 128, w).transpose(1, 0, 2).reshape(128, 8 * w)
        pieces.append(p)
    wqkvS = np.concatenate(pieces, axis=1)  # [128, 6144]
    return {
        "xT": np.ascontiguousarray(x[b].T).astype(bf16),
        "wqkvS": np.ascontiguousarray(wqkvS).astype(bf16),
        "bqk": np.ascontiguousarray(
            np.stack([bq[0:128], bq[128:256], bk[0:128], bk[128:256]], axis=1)
        ),
        "bv_rep": np.ascontiguousarray(np.broadcast_to(bv, (128, 256))),
        "wfcT": np.ascontiguousarray(w_fc[:, 256 * g : 256 * g + 256].T).astype(bf16),
        "mask": np.triu(np.ones((128, 128), dtype=np.float32)).astype(bf16),
    }


def kernel(x, w_qkv, b_qkv, w_fc, b_fc):
    global LAST_RESULT
    x = np.asarray(x, dtype=np.float32)
    w_qkv = np.asarray(w_qkv, dtype=np.float32)
    b_qkv = np.asarray(b_qkv, dtype=np.float32)
    w_fc = np.asarray(w_fc, dtype=np.float32)
    b_fc = np.asarray(b_fc, dtype=np.float32)

    if "nc" not in _BUILD_CACHE:
        _BUILD_CACHE["nc"] = build()
    nc = _BUILD_CACHE["nc"]

    in_maps = [
        _prep_core_inputs(x, w_qkv, b_qkv, w_fc, b_fc, core) for core in range(8)
    ]
    res = run_bass_kernel_spmd(
        nc,
        in_maps,
        core_ids=list(range(8)),
        trace=bool(os.environ.get("MHA_TRACE")),
    )
    LAST_RESULT = res

    out = np.empty((B, T, C), dtype=np.float32)
    for b in range(B):
        acc = res.results[4 * b]["y"].astype(np.float32)
        acc[T - TCH :] += res.results[4 * b]["y2"].astype(np.float32)
        for g in range(1, 4):
            acc = acc + res.results[4 * b + g]["y"].astype(np.float32)
            acc[T - TCH :] += res.results[4 * b + g]["y2"].astype(np.float32)
        out[b] = acc + b_fc
    return out
